# revision 4
# baseline (speedup 1.0000x reference)
"""Balanced-softmax loss (BSLClassifier) on 8 Trainium2 NeuronCores.

loss = -(1/B) * sum_b [ pred[b,t_b] + log(freq[t_b]) - log(sum_c exp(pred[b,c])*freq[c]) ]

Strategy (data-parallel over batch):
  - host: global histogram freq[c] of targets (tiny), shard pred/target by batch
  - device, per core, streaming 32 tiles of [128 rows x 1000 classes]:
      ACT : expT = exp(pred_tile)                      (fp32 -> bf16)
      DVE : tensor_tensor_reduce  expT * freq  -> rsum accum (fp32)   [2x bf16 mode]
      DVE : scalar_tensor_tensor (iota == t_p) * expT -> picked accum (fp32)
  - host: loss = -(1/B) * [ sum log(picked) + sum log(freq[t]) - sum log(rsum) ]  (f64)

pred is read exactly once from HBM -> memory-bound at the per-core HBM roofline.
"""

import numpy as np
import ml_dtypes

B, C = 32768, 1000
NCORES = 8
ROWS = B // NCORES  # 4096 rows per core
P = 128             # partitions
NT = ROWS // P      # 32 tiles per core

_CACHE = {}


def _split_multi_waits(nc, max_waits=1):
    """This container's walrus build accepts at most one sync-wait per
    instruction; Tile emits several. Split extras into standalone
    EventSemaphore instructions on the same engine, immediately before."""
    from concourse import mybir

    n_new = 0
    for func in nc.m.functions:
        for bb in func.blocks:
            out = []
            changed = False
            for ins in bb.instructions:
                si = ins.sync_info
                if si is not None and len(si.on_wait) > max_waits:
                    waits = list(si.on_wait)
                    extra, keep = waits[:-max_waits], waits[-max_waits:]
                    for w in extra:
                        n_new += 1
                        ev = mybir.InstEventSemaphore(
                            name=f"wsplit_{n_new}", ins=[], outs=[]
                        )
                        ev.engine = ins.engine
                        ev.sync_info = mybir.SyncInfo(on_update=[], on_wait=[w])
                        out.append(ev)
                    ins.sync_info = mybir.SyncInfo(
                        on_update=list(si.on_update), on_wait=keep
                    )
                    changed = True
                out.append(ins)
            if changed:
                bb.instructions = out
    return n_new


def _build_bass():
    import concourse.bass as bass
    import concourse.tile as tile
    from concourse import mybir

    f32 = mybir.dt.float32
    bf16 = mybir.dt.bfloat16
    i16 = mybir.dt.int16
    Alu = mybir.AluOpType
    Act = mybir.ActivationFunctionType

    nc = bass.Bass()
    pred = nc.dram_tensor("pred", [ROWS, C], f32, kind="ExternalInput")
    freqb = nc.dram_tensor("freqb", [P, C], bf16, kind="ExternalInput")
    iotab = nc.dram_tensor("iotab", [P, C], i16, kind="ExternalInput")
    tgt = nc.dram_tensor("tgt", [P, NT], i16, kind="ExternalInput")
    pexp = nc.dram_tensor("pexp", [P, NT], f32, kind="ExternalOutput")
    rsum = nc.dram_tensor("rsum", [P, NT], f32, kind="ExternalOutput")

    pred_r = pred[:].rearrange("(n p) c -> n p c", p=P)

    with tile.TileContext(nc) as tc:
        with (
            tc.tile_pool(name="const", bufs=1) as const_pool,
            tc.tile_pool(name="io", bufs=4) as io_pool,
            tc.tile_pool(name="work", bufs=3) as work_pool,
            tc.tile_pool(name="acc", bufs=1) as acc_pool,
        ):
            freq_t = const_pool.tile([P, C], bf16)
            nc.sync.dma_start(out=freq_t, in_=freqb[:])
            iota_t = const_pool.tile([P, C], i16)
            nc.sync.dma_start(out=iota_t, in_=iotab[:])
            tgt_t = const_pool.tile([P, NT], i16)
            nc.sync.dma_start(out=tgt_t, in_=tgt[:])

            pexp_acc = acc_pool.tile([P, NT], f32)
            rsum_acc = acc_pool.tile([P, NT], f32)

            for t in range(NT):
                ptile = io_pool.tile([P, C], f32, tag="ptile")
                nc.sync.dma_start(out=ptile, in_=pred_r[t])

                expt = work_pool.tile([P, C], bf16, tag="expt")
                nc.scalar.activation(expt, ptile, Act.Exp)

                scr1 = work_pool.tile([P, C], bf16, tag="scr1")
                nc.vector.scalar_tensor_tensor(
                    out=scr1,
                    in0=expt,
                    scalar=1.0,
                    in1=freq_t,
                    op0=Alu.mult,
                    op1=Alu.mult,
                    accum_out=rsum_acc[:, t : t + 1],
                )

                scr2 = work_pool.tile([P, C], bf16, tag="scr2")
                nc.vector.scalar_tensor_tensor(
                    out=scr2,
                    in0=iota_t,
                    scalar=tgt_t[:, t : t + 1],
                    in1=expt,
                    op0=Alu.is_equal,
                    op1=Alu.mult,
                    accum_out=pexp_acc[:, t : t + 1],
                )

            nc.sync.dma_start(out=pexp[:], in_=pexp_acc)
            nc.sync.dma_start(out=rsum[:], in_=rsum_acc)

    _split_multi_waits(nc)
    return nc


def kernel(pred, target):
    from concourse.bass_utils import run_bass_kernel_spmd

    pred = np.ascontiguousarray(np.asarray(pred), dtype=np.float32)
    target = np.asarray(target)
    tgt64 = target.astype(np.int64)
    assert pred.shape == (B, C) and tgt64.shape == (B,)

    if "nc" not in _CACHE:
        _CACHE["nc"] = _build_bass()
    nc = _CACHE["nc"]

    # host-side tiny index math
    freq = np.bincount(tgt64, minlength=C).astype(np.float64)
    freqb = np.ascontiguousarray(
        np.broadcast_to(freq.astype(ml_dtypes.bfloat16), (P, C))
    )
    iotab = np.ascontiguousarray(
        np.broadcast_to(np.arange(C, dtype=np.int16), (P, C))
    )

    in_maps = []
    for c in range(NCORES):
        sl = slice(c * ROWS, (c + 1) * ROWS)
        tgt_c = np.ascontiguousarray(
            tgt64[sl].reshape(NT, P).T.astype(np.int16)
        )
        in_maps.append(
            {
                "pred": pred[sl],
                "freqb": freqb,
                "iotab": iotab,
                "tgt": tgt_c,
            }
        )

    res = run_bass_kernel_spmd(nc, in_maps, core_ids=list(range(NCORES)))
    _CACHE["last_results"] = res

    # host-side final reduction in f64 (tiny: 2 * 32768 values)
    s = 0.0
    logfreq = np.log(np.maximum(freq, 1.0))  # f64; freq[t_b] >= 1 always
    s += logfreq[tgt64].sum()
    for c in range(NCORES):
        out = res.results[c]
        s += np.log(out["pexp"].astype(np.float64)).sum()
        s -= np.log(out["rsum"].astype(np.float64)).sum()
    return np.asarray(-s / B, dtype=np.float32)


# revision 18
# speedup vs baseline: 1.2393x; 1.2393x over previous
"""Balanced-softmax loss (BSLClassifier) on 8 Trainium2 NeuronCores.

loss = -(1/B) * sum_b [ pred[b,t_b] + log(freq[t_b]) - log(sum_c exp(pred[b,c])*freq[c]) ]

Strategy: data-parallel over batch B; each core's shard is laid out
class-major ([C=1000, Bc=4096], a host-side layout choice) so that:
  - ACT : exp(pred_T + logfreq) in one op -- logfreq[c] is constant per
          partition, so it rides the activation's per-partition bias.
          Output expT bf16.
  - PE  : rsum[b] = sum_c exp(...) via ones-vector matvecs in bf16
          (contract over the 128-class partition dim, accumulate the 8
          class chunks in PSUM, fp32).
  - DVE : picked = sum_b pred_T[t_b, b] via one fused
          scalar_tensor_tensor per chunk: (t_row == iota_c) * pred_T,
          free-dim accumulate (fp32, exact gather).
  - host: histogram, tiny log/sum finalization in f64.

pred is read exactly once from HBM -> memory-bound at the per-core HBM
roofline; ACT/PE/DVE each stay below the DMA time.
"""

import numpy as np
import ml_dtypes

B, C = 32768, 1000
NCORES = 8
BC = B // NCORES    # 4096 batch columns per core
P = 128             # partitions
NK = (C + P - 1) // P  # 8 class chunks (last one 104 rows)
NJ = BC // 512      # 8 psum column blocks per core

_CACHE = {}


def _split_multi_waits(nc, max_waits=1):
    """This container's walrus build accepts at most one sync-wait per
    instruction; Tile emits several. Split extras into standalone
    EventSemaphore instructions on the same engine, immediately before."""
    from concourse import mybir

    n_new = 0
    for func in nc.m.functions:
        for bb in func.blocks:
            out = []
            changed = False
            for ins in bb.instructions:
                si = ins.sync_info
                if si is not None and len(si.on_wait) > max_waits:
                    waits = list(si.on_wait)
                    extra, keep = waits[:-max_waits], waits[-max_waits:]
                    for w in extra:
                        n_new += 1
                        ev = mybir.InstEventSemaphore(
                            name=f"wsplit_{n_new}", ins=[], outs=[]
                        )
                        ev.engine = ins.engine
                        ev.sync_info = mybir.SyncInfo(on_update=[], on_wait=[w])
                        out.append(ev)
                    ins.sync_info = mybir.SyncInfo(
                        on_update=list(si.on_update), on_wait=keep
                    )
                    changed = True
                out.append(ins)
            if changed:
                bb.instructions = out
    return n_new


def _build_bass():
    import concourse.bass as bass
    import concourse.tile as tile
    from concourse import mybir

    f32 = mybir.dt.float32
    bf16 = mybir.dt.bfloat16
    i16 = mybir.dt.int16
    Alu = mybir.AluOpType
    Act = mybir.ActivationFunctionType

    nc = bass.Bass()
    predt = nc.dram_tensor("predt", [C, BC], f32, kind="ExternalInput")
    lfcol = nc.dram_tensor("lfcol", [P, NK], f32, kind="ExternalInput")
    tbc = nc.dram_tensor("tbc", [P, BC], i16, kind="ExternalInput")
    iotac = nc.dram_tensor("iotac", [P, NK], i16, kind="ExternalInput")
    onesb = nc.dram_tensor("onesb", [P, 1], bf16, kind="ExternalInput")
    rsum = nc.dram_tensor("rsum", [1, BC], f32, kind="ExternalOutput")
    picked = nc.dram_tensor("picked", [P, NK], f32, kind="ExternalOutput")

    with tile.TileContext(nc) as tc:
        with (
            tc.tile_pool(name="const", bufs=1) as const_pool,
            tc.tile_pool(name="io", bufs=3) as io_pool,
            tc.tile_pool(name="work", bufs=2) as work_pool,
            tc.tile_pool(name="ps", bufs=1, space="PSUM") as psum_pool,
            tc.tile_pool(name="acc", bufs=1) as acc_pool,
        ):
            lf_t = const_pool.tile([P, NK], f32)
            nc.sync.dma_start(out=lf_t, in_=lfcol[:])
            iota_t = const_pool.tile([P, NK], i16)
            nc.sync.dma_start(out=iota_t, in_=iotac[:])
            ones_t = const_pool.tile([P, 1], bf16)
            nc.sync.dma_start(out=ones_t, in_=onesb[:])
            tbc_t = const_pool.tile([P, BC], i16)
            nc.sync.dma_start(out=tbc_t, in_=tbc[:])

            picked_acc = acc_pool.tile([P, NK], f32)
            # one bank per 512-column block, all on partition 0
            rsum_ps = psum_pool.tile([1, NJ, 512], f32)

            for k in range(NK):
                pk = min(P, C - k * P)  # 104 on the last chunk
                ptile = io_pool.tile([P, BC], f32, tag="ptile")
                nc.sync.dma_start(
                    out=ptile[:pk], in_=predt[k * P : k * P + pk, :]
                )

                expt = work_pool.tile([P, BC], bf16, tag="expt")
                nc.scalar.activation(
                    expt[:pk], ptile[:pk], Act.Exp, bias=lf_t[:pk, k : k + 1]
                )

                for j in range(NJ):
                    nc.tensor.matmul(
                        rsum_ps[0:1, j, :],
                        ones_t[:pk],
                        expt[:pk, j * 512 : (j + 1) * 512],
                        start=(k == 0),
                        stop=(k == NK - 1),
                    )

                scr = work_pool.tile([P, BC], bf16, tag="scr")
                nc.vector.scalar_tensor_tensor(
                    out=scr[:pk],
                    in0=tbc_t[:pk],
                    scalar=iota_t[:pk, k : k + 1],
                    in1=ptile[:pk],
                    op0=Alu.is_equal,
                    op1=Alu.mult,
                    accum_out=picked_acc[:pk, k : k + 1],
                )

            rsum_sb = acc_pool.tile([1, BC], f32)
            nc.scalar.copy(
                rsum_sb[:].rearrange("p (j b) -> p j b", j=NJ), rsum_ps[:, :, :]
            )
            nc.sync.dma_start(out=rsum[:], in_=rsum_sb)
            nc.sync.dma_start(out=picked[:], in_=picked_acc)

    _split_multi_waits(nc)
    return nc


def kernel(pred, target):
    from concourse.bass_utils import run_bass_kernel_spmd

    pred = np.asarray(pred)
    target = np.asarray(target)
    tgt64 = target.astype(np.int64)
    assert pred.shape == (B, C) and tgt64.shape == (B,)

    if "nc" not in _CACHE:
        _CACHE["nc"] = _build_bass()
    nc = _CACHE["nc"]

    # host-side tiny index math
    freq = np.bincount(tgt64, minlength=C).astype(np.float64)
    logfreq = np.where(freq > 0, np.log(np.maximum(freq, 1.0)), -30000.0)
    lf32 = logfreq.astype(np.float32)
    lfcol = np.zeros((P, NK), dtype=np.float32)
    lfcol.reshape(-1)[:C] = 0  # layout: lfcol[p, k] = lf[k*P + p]
    iotac = np.zeros((P, NK), dtype=np.int16)
    for k in range(NK):
        pk = min(P, C - k * P)
        lfcol[:pk, k] = lf32[k * P : k * P + pk]
        iotac[:pk, k] = np.arange(k * P, k * P + pk, dtype=np.int16)
    onesb = np.ones((P, 1), dtype=ml_dtypes.bfloat16)

    in_maps = []
    for c in range(NCORES):
        sl = slice(c * BC, (c + 1) * BC)
        predt_c = np.ascontiguousarray(pred[sl].T.astype(np.float32, copy=False))
        tbc_c = np.ascontiguousarray(
            np.broadcast_to(tgt64[sl].astype(np.int16), (P, BC))
        )
        in_maps.append(
            {
                "predt": predt_c,
                "lfcol": lfcol,
                "tbc": tbc_c,
                "iotac": iotac,
                "onesb": onesb,
            }
        )

    res = run_bass_kernel_spmd(nc, in_maps, core_ids=list(range(NCORES)))
    _CACHE["last_results"] = res

    # host-side final reduction in f64 (tiny)
    # picked sums pred[b, t_b] (exact fp32); rsum[b] = sum_c exp(pred+lf)
    s = 0.0
    s += logfreq[tgt64].sum()  # sum_b log(freq[t_b])
    for c in range(NCORES):
        out = res.results[c]
        s += out["picked"].astype(np.float64).sum()
        s -= np.log(out["rsum"].astype(np.float64)).sum()
    return np.asarray(-s / B, dtype=np.float32)
